# revision 27
# baseline (speedup 1.0000x reference)
"""Trainium2 Bass kernel for nn_MultiHeadAttention (B=2, S=2048, D=2048, H=16, HD=128).

Sharding: tensor-parallel across heads. Each of the 8 cores owns 2 heads:
  - QKV projection for its heads (bf16 matmuls, fp32 PSUM accumulation,
    K-contiguous)
  - RoPE (DVE, fp32 math on PSUM) with host-precomputed cos / sign-folded sin
  - causal attention in transposed layout scores^T[k, q]:
      exp via ScalarE (scale = HD^-0.5 folded in), causal mask by multiplying
      with a sliced 0/1 mask, softmax denominator via ones-vector matmul on PE,
      attn @ V accumulated as out^T[d, q] (moving dim 512 throughout)
  - softmax normalization is software-pipelined one chunk behind the matmul
    stream so the slow DVE reciprocal never stalls the PE queue; heads are
    interleaved (c outer, h inner) so each reciprocal hides under the other
    head's matmul window
  - normalized chunks are DMA-staged into the AllToAll input as they are
    produced; one AllToAll per batch redistributes head-outputs into
    sequence-slices (bf16)
  - output projection W_o with weights prefetched into SBUF during the
    (DMA-idle) attention phase
Host gathers by concatenating the 8 row-slices.
"""

import numpy as np

B = 2
H = 16
HD = 128
N_CORES = 8
HEADS_PER_CORE = H // N_CORES


# ---------------------------------------------------------------- device code
def build_nc(S=2048, D=2048, n_cores=N_CORES):
    from contextlib import ExitStack

    import concourse.mybir as mybir
    import concourse.tile as tile
    from concourse import bacc

    f32 = mybir.dt.float32
    f32r = mybir.dt.float32r
    bf16 = mybir.dt.bfloat16
    fp16 = mybir.dt.float16
    Exp = mybir.ActivationFunctionType.Exp

    KT = D // 128            # contraction tiles for projections
    S2 = B * S               # total rows of x
    NCH = S // 512           # 512-wide q-chunks per batch
    SLICE = S2 // n_cores    # output rows per core
    ECH = D // 512           # 512-wide e-chunks of the output dim
    HSC = HEADS_PER_CORE
    hd_scale = float(HD) ** -0.5

    nc = bacc.Bacc("TRN2", target_bir_lowering=False, debug=False,
                   num_devices=n_cores)

    xt = nc.dram_tensor("xt", [D, S2], bf16, kind="ExternalInput").ap()
    wqk = nc.dram_tensor("wqk", [D, 2 * HSC * 128], bf16, kind="ExternalInput").ap()
    wv = nc.dram_tensor("wv", [D, HSC * 128], bf16, kind="ExternalInput").ap()
    wo = nc.dram_tensor("wo", [H * HD, D], bf16, kind="ExternalInput").ap()
    cost = nc.dram_tensor("cost", [128, S], bf16, kind="ExternalInput").ap()
    sins = nc.dram_tensor("sins", [128, S], bf16, kind="ExternalInput").ap()
    maskt = nc.dram_tensor("maskt", [128, 896], bf16, kind="ExternalInput").ap()
    ones = nc.dram_tensor("ones", [128, 1], bf16, kind="ExternalInput").ap()
    onesr = nc.dram_tensor("onesr", [1, 128], f32r, kind="ExternalInput").ap()
    out = nc.dram_tensor("out", [SLICE, D], fp16, kind="ExternalOutput").ap()

    xt_rs = xt.rearrange("(kt p) s -> p kt s", p=128)
    wqk_rs = wqk.rearrange("(kt p) m -> p kt m", p=128)
    wv_rs = wv.rearrange("(kt p) m -> p kt m", p=128)
    wo_rs = wo.rearrange("(ht p) e -> p ht e", p=128)

    with tile.TileContext(nc) as tc, ExitStack() as ctx:
        const = ctx.enter_context(tc.tile_pool(name="const", bufs=1))
        stream = ctx.enter_context(tc.tile_pool(name="stream", bufs=4))
        atnp = ctx.enter_context(tc.tile_pool(name="atnp", bufs=2))
        qkvp = ctx.enter_context(tc.tile_pool(name="qkvp", bufs=1))
        expp = ctx.enter_context(tc.tile_pool(name="expp", bufs=2))
        accp = ctx.enter_context(tc.tile_pool(name="accp", bufs=1))
        tmp = ctx.enter_context(tc.tile_pool(name="tmp", bufs=2))
        stg = ctx.enter_context(tc.tile_pool(name="stg", bufs=2))
        psA = ctx.enter_context(tc.tile_pool(name="psA", bufs=3, space="PSUM"))
        psB = ctx.enter_context(tc.tile_pool(name="psB", bufs=2, space="PSUM"))
        psC = ctx.enter_context(tc.tile_pool(name="psC", bufs=1, space="PSUM"))
        psD = ctx.enter_context(tc.tile_pool(name="psD", bufs=2, space="PSUM"))
        dram = ctx.enter_context(tc.tile_pool(name="dram", bufs=1, space="DRAM"))

        # resident constants. DMA enqueue order is chosen so that the first
        # QKV matmul (needs wqk + x chunk 0) is gated by as little traffic
        # as possible: wqk half -> xt chunk 0 half -> ... (the kt loop
        # consumes contraction tiles in order, so each half-load unblocks
        # half the matmuls)
        KTH = KT // 2
        wqk_sb = const.tile([128, KT, 2 * HSC * 128], bf16)
        nc.sync.dma_start(wqk_sb[:, :KTH], wqk_rs[:, :KTH])
        wv_sb = const.tile([128, KT, HSC * 128], bf16)
        cos_sb = const.tile([128, S], bf16)
        sins_sb = const.tile([128, S], bf16)
        mask_sb = const.tile([128, 896], bf16)
        ones_sb = const.tile([128, 1], bf16)
        onesr_sb = const.tile([1, 128], f32r)
        wo_sb = const.tile([128, H, D], bf16)

        SL8 = S // n_cores
        a2a_outs = [[None] * HSC for _ in range(B)]
        a2a_ins = []
        atn_sbs = [atnp.tile([128, HSC, n_cores, SL8], bf16, name=f"atn_{bi}")
                   for bi in range(B)]

        # deferred softmax-normalize work items:
        #   (b, h, c, av_tile, rs_tile, a_in_tile)
        pending = []

        def emit_collective(bi, h):
            # one AllToAll per (batch, head): the h=0 transfer overlaps the
            # h=1 tail, and the W_o-side gather starts as soon as possible
            a_out = dram.tile([n_cores, 128, SL8], bf16,
                              name=f"a2a_out_{bi}_{h}")
            nc.gpsimd.collective_compute(
                "AllToAll",
                mybir.AluOpType.bypass,
                replica_groups=[list(range(n_cores))],
                ins=[a2a_ins[bi][h].opt()],
                outs=[a_out.opt()],
            )
            a2a_outs[bi][h] = a_out
            # gather into this core's W_o operand as [p, h, r, s]
            nc.sync.dma_start(atn_sbs[bi][:, h],
                              a_out.rearrange("r p s -> p r s"))

        def emit_norm(item):
            b, h, c, av, rs_rcp, a_in = item
            # partition-broadcast 1/rowsum via PE outer product
            bc = psA.tile([128, 512], f32, tag="psA")
            nc.tensor.matmul(bc, onesr_sb[:], rs_rcp)
            bcs = tmp.tile([128, 512], f32, tag="bcs")
            nc.scalar.copy(bcs[:], bc[:])
            st = stg.tile([128, 512], bf16, tag="stg")
            nc.vector.tensor_mul(st[:], av, bcs[:])
            # stage straight into this batch's AllToAll input: chunk c of
            # head h covers destination cores 2c and 2c+1
            for i in range(2):
                nc.sync.dma_start(a_in[2 * c + i],
                                  st[:, i * SL8:(i + 1) * SL8])

        STB = max(1, SL8 // 128)
        PS = min(128, SL8)

        def emit_po(b, ec, st, pool, tag):
            po = pool.tile([128, 512], f32, tag=tag,
                           name=f"po_{b}_{ec}_{st}")[:PS]
            atn_sb = atn_sbs[b]
            for h in range(HSC):
                for r in range(n_cores):
                    nc.tensor.matmul(
                        po,
                        atn_sb[:, h, r, st * 128:st * 128 + PS],
                        wo_sb[:, r * HSC + h, ec * 512:(ec + 1) * 512],
                        start=(h == 0 and r == 0),
                        stop=(h == HSC - 1 and r == n_cores - 1),
                    )
            ot = tmp.tile([128, 512], fp16, tag="bcs", name=f"ot_{b}_{ec}_{st}")
            nc.scalar.copy(ot[:PS, :], po)
            r0 = b * SL8 + st * 128
            for j in range(2):  # two half-width writes spread across queues
                e0 = ec * 512 + j * 256
                nc.sync.dma_start(out[r0:r0 + PS, e0:e0 + 256],
                                  ot[:PS, j * 256:(j + 1) * 256])

        def drain(item):
            emit_norm(item)
            b, h, c = item[0], item[1], item[2]
            if c == NCH - 1:  # head h fully staged: fire its AllToAll
                emit_collective(b, h)

        for b in range(B):
            # -------- QKV projection for batch b (heads of this core) --------
            q_sb = [qkvp.tile([128, S], bf16, tag=f"q{h}", name=f"q{h}_{b}")
                    for h in range(HSC)]
            k_sb = [qkvp.tile([128, S], bf16, tag=f"k{h}", name=f"k{h}_{b}")
                    for h in range(HSC)]
            v_sb = qkvp.tile([128, S // 128, HSC * 128], bf16, tag="v")

            for e4 in range(S // 512):
                s0 = e4 * 512
                xcol = xt_rs[:, :, b * S + s0:b * S + s0 + 512]
                xt_h = [stream.tile([128, KTH, 512], bf16, tag="stream",
                                    name=f"xt_{b}_{e4}_{i}")
                        for i in range(2)]
                nc.sync.dma_start(xt_h[0][:], xcol[:, :KTH])
                if b == 0 and e4 == 0:
                    nc.sync.dma_start(wqk_sb[:, KTH:], wqk_rs[:, KTH:])
                nc.sync.dma_start(xt_h[1][:], xcol[:, KTH:])
                if b == 0:
                    if e4 == 0:
                        nc.sync.dma_start(wv_sb[:], wv_rs[:])
                    # cos/sin arrive in per-chunk slices right behind the x
                    # chunk that needs them
                    nc.sync.dma_start(cos_sb[:, s0:s0 + 512], cost[:, s0:s0 + 512])
                    nc.sync.dma_start(sins_sb[:, s0:s0 + 512], sins[:, s0:s0 + 512])
                    if e4 == 0:
                        nc.sync.dma_start(mask_sb[:], maskt[:])
                        nc.sync.dma_start(ones_sb[:], ones[:])
                        nc.sync.dma_start(onesr_sb[:], onesr[:])

                # q/k tiles: out^T layout [c, s], N=512. ct pairs with a
                # kt-half-major loop so the first matmuls only gate on the
                # first half-loads of wqk and the x chunk.
                for cp in range(HSC):
                    pss = [psA.tile([128, 512], f32, tag="psA", name="ps_qk")
                           for _ in range(2)]
                    for kth in range(2):
                        for ci in range(2):
                            ct = 2 * cp + ci
                            for kt in range(kth * KTH, (kth + 1) * KTH):
                                nc.tensor.matmul(
                                    pss[ci],
                                    wqk_sb[:, kt, ct * 128:(ct + 1) * 128],
                                    xt_h[kt // KTH][:, kt % KTH, :],
                                    start=(kt == 0), stop=(kt == KT - 1),
                                )
                    for ci in range(2):
                        ct = 2 * cp + ci
                        ps = pss[ci]
                        dst = q_sb[ct] if ct < HSC else k_sb[ct - HSC]
                        sl = slice(s0, s0 + 512)
                        # RoPE: dst = ps*cos + swap_half(ps)*sign_sin
                        t1 = tmp.tile([128, 512], f32, tag="ropetmp")
                        nc.vector.tensor_mul(t1[:], ps, cos_sb[:, sl])
                        t2 = tmp.tile([128, 512], f32, tag="ropetmp2")
                        nc.vector.tensor_mul(t2[0:64, :], ps[64:128, :], sins_sb[0:64, sl])
                        nc.vector.tensor_mul(t2[64:128, :], ps[0:64, :], sins_sb[64:128, sl])
                        nc.vector.tensor_add(dst[:, sl], t1[:], t2[:])

                # v tiles: natural [s, c] layout
                for sv in range(4):
                    psv = psC.tile([128, HSC * 128], f32, tag="psC")
                    for kt in range(KT):
                        nc.tensor.matmul(
                            psv,
                            xt_h[kt // KTH][:, kt % KTH, sv * 128:(sv + 1) * 128],
                            wv_sb[:, kt, :],
                            start=(kt == 0), stop=(kt == KT - 1),
                        )
                    nc.scalar.copy(v_sb[:, e4 * 4 + sv, :], psv[:])

                # flush any deferred normalize work from the previous batch's
                # attention under this chunk's matmul window (fires that
                # batch's remaining AllToAll once its last chunk is staged)
                while pending:
                    drain(pending.pop(0))

                if b == 1 and e4 == 1:
                    # W_o weights stream during the DMA-idle attention phase
                    nc.sync.dma_start(wo_sb[:], wo_rs[:])

            # -------- causal attention for batch b --------
            # head-interleaved (c outer, h inner) with normalization deferred
            # by one (c, h) step: the DVE reciprocal of one chunk runs under
            # the other head's matmul window, so the PE never waits on it.
            a_in = [dram.tile([n_cores, 128, SL8], bf16, name=f"a2a_in_{b}_{h}")
                    for h in range(HSC)]
            a2a_ins.append(a_in)
            # For the last batch, the per-kt PE rowsum is replaced by DVE
            # fp16 accumulation of the exp tiles (one PE rowsum per chunk),
            # and the freed PE time is filled with batch-0 W_o blocks
            # (whose AllToAll results are already resident).
            wo_blocks = ([(0, ec, st) for ec in range(ECH)
                          for st in range(STB)] if b == B - 1 else [])
            for c in range(NCH):
                for h in range(HSC):
                    qh, kh = q_sb[h], k_sb[h]
                    nkt = 4 * c + 4
                    offload = (b == B - 1)
                    av = psB.tile([128, 512], f32, tag="psB")
                    rs = psD.tile([1, 512], f32, tag="psD")
                    acc = None
                    if offload:
                        acc = accp.tile([128, 512], fp16, tag="acc",
                                        name=f"acc_{c}_{h}")
                    ex_prev = None
                    for kt in range(nkt):
                        sc = psA.tile([128, 512], f32, tag="psA")
                        nc.tensor.matmul(
                            sc,
                            kh[:, kt * 128:(kt + 1) * 128],
                            qh[:, c * 512:(c + 1) * 512],
                        )
                        ex = expp.tile([128, 512], fp16)
                        nc.scalar.activation(ex[:], sc[:], Exp, scale=hd_scale)
                        if kt >= 4 * c:  # diagonal band: causal mask
                            j0 = 384 - (kt * 128 - c * 512)
                            nc.vector.tensor_mul(ex[:], ex[:], mask_sb[:, j0:j0 + 512])
                        nc.tensor.matmul(
                            av,
                            v_sb[:, kt, h * 128:(h + 1) * 128],
                            ex[:],
                            start=(kt == 0), stop=(kt == nkt - 1),
                        )
                        if not offload:
                            nc.tensor.matmul(
                                rs,
                                ones_sb[:],
                                ex[:],
                                start=(kt == 0), stop=(kt == nkt - 1),
                            )
                        elif kt == 1:
                            nc.vector.tensor_add(acc[:], ex_prev[:], ex[:])
                        elif kt >= 2:
                            nc.vector.tensor_add(acc[:], acc[:], ex[:])
                        ex_prev = ex
                    if offload:
                        # single partition-sum of the accumulated exp tiles
                        nc.tensor.matmul(rs, ones_sb[:], acc[:])
                    # 1/rowsum immediately (off the PE path); broadcast +
                    # normalize deferred until the next chunk's matmuls are
                    # enqueued
                    rcp = tmp.tile([1, 512], f32r, tag="rcp")
                    with nc.allow_low_precision(reason="fp32r softmax denom"):
                        nc.vector.reciprocal(rcp[:], rs[:])
                    if pending:
                        drain(pending.pop(0))
                    pending.append((b, h, c, av, rcp, a_in[h]))
                    # feed two batch-0 W_o blocks into the PE stream (skip
                    # the first iteration so the gathers are resident)
                    if wo_blocks and not (c == 0 and h == 0):
                        for _ in range(2):
                            if len(wo_blocks) > 2:
                                bb, ec, st = wo_blocks.pop(0)
                                emit_po(bb, ec, st, psC, 'psC')

            if b == B - 1:
                # last batch: no following matmul window to hide behind
                while pending:
                    drain(pending.pop(0))
                # leftover batch-0 W_o blocks cover the last AllToAll wait
                while wo_blocks:
                    bb, ec, st = wo_blocks.pop(0)
                    emit_po(bb, ec, st, psB, 'psB')

        # -------- output projection, batch-1 rows (batch 0 was emitted
        # interleaved with the batch-1 attention stream) --------
        for ec in range(ECH):
            for st in range(STB):
                emit_po(1, ec, st, psB, 'psB')

    nc.finalize()
    return nc


# ---------------------------------------------------------------- host code
def round_fp32r(a):
    """Round float32 to the PE's FP32R format: round-to-nearest-even to an
    11-bit mantissa (low 12 mantissa bits zero)."""
    b = np.ascontiguousarray(a, dtype=np.float32).view(np.uint32)
    bias = np.uint32(0x7FF) + ((b >> np.uint32(12)) & np.uint32(1))
    r = (b + bias) & np.uint32(0xFFFFF000)
    return r.view(np.float32)


def make_tables(S):
    half = HD // 2
    inv_freq = (1.0 / (10000.0 ** (np.arange(half, dtype=np.float32) / half)))
    pos = np.arange(S, dtype=np.float32)
    freqs = pos[:, None] * inv_freq[None, :]          # [S, half]
    cos = np.cos(freqs).astype(np.float32)            # [S, half]
    sin = np.sin(freqs).astype(np.float32)
    cosT = np.concatenate([cos, cos], axis=1).T       # [HD, S]
    # sign-folded sin: rows 0..63 get -sin, rows 64..127 get +sin
    sinsT = np.concatenate([-sin, sin], axis=1).T     # [HD, S]
    return np.ascontiguousarray(cosT), np.ascontiguousarray(sinsT)


def make_mask():
    j = np.arange(896)[None, :]
    k = np.arange(128)[:, None]
    return ((j - 384) >= k).astype(np.float32)        # [128, 896]


def prepare_in_maps(x, W_qkv, W_o, S, D):
    import ml_dtypes
    bf16 = ml_dtypes.bfloat16

    S2 = B * S
    xT = np.ascontiguousarray(
        x.reshape(S2, D).T.astype(np.float32)).astype(bf16)
    cosT, sinsT = make_tables(S)
    mask = make_mask()
    ones = np.ones((128, 1), bf16)
    onesr = np.ones((1, 128), np.float32)
    wo_bf16 = W_o.astype(bf16)

    qw = W_qkv[:, 0 * H * HD:1 * H * HD]
    kw = W_qkv[:, 1 * H * HD:2 * H * HD]
    vw = W_qkv[:, 2 * H * HD:3 * H * HD]

    in_maps = []
    for c in range(N_CORES):
        h0 = c * HEADS_PER_CORE
        cols = slice(h0 * HD, (h0 + HEADS_PER_CORE) * HD)
        wqk_c = np.ascontiguousarray(
            np.concatenate([qw[:, cols], kw[:, cols]], axis=1)).astype(bf16)
        wv_c = np.ascontiguousarray(vw[:, cols]).astype(bf16)
        in_maps.append({
            "xt": xT, "wqk": wqk_c, "wv": wv_c, "wo": wo_bf16,
            "cost": cosT.astype(bf16), "sins": sinsT.astype(bf16),
            "maskt": mask.astype(bf16),
            "ones": ones, "onesr": onesr,
        })
    return in_maps


_NC_CACHE = {}


def run(x, W_qkv, W_o, S, D, trace=False, trace_kwargs=None):
    from concourse.bass_utils import run_bass_kernel_spmd

    key = (S, D)
    if key not in _NC_CACHE:
        _NC_CACHE[key] = build_nc(S=S, D=D)
    nc = _NC_CACHE[key]
    in_maps = prepare_in_maps(x, W_qkv, W_o, S, D)
    res = run_bass_kernel_spmd(
        nc, in_maps, core_ids=list(range(N_CORES)),
        trace=trace, **(trace_kwargs or {}),
    )
    SL8 = S // N_CORES
    full = np.empty((B, S, D), np.float32)
    for c in range(N_CORES):
        o = np.asarray(res.results[c]["out"], dtype=np.float32)
        full[0, c * SL8:(c + 1) * SL8] = o[:SL8]
        full[1, c * SL8:(c + 1) * SL8] = o[SL8:]
    return full, res


def kernel(x, W_qkv, W_o):
    x = np.asarray(x)
    W_qkv = np.asarray(W_qkv)
    W_o = np.asarray(W_o)
    S, D = x.shape[1], x.shape[2]
    out, _ = run(x, W_qkv, W_o, S, D, trace=False)
    return out.astype(np.float32)
